# revision 2
# baseline (speedup 1.0000x reference)
"""Trainium2 Bass kernel for a 2-layer GatedGraphConv encoder (9 convs, 18
message-passing + GRU steps) on N=50000 nodes, E=800000 edges, C=128.

Strategy (8 NeuronCores, SPMD single program):
  - Nodes are block-sharded: core c owns dst rows [c*6250, (c+1)*6250).
  - Everything stays fp32: the GRU iteration amplifies injected noise
    ~x1.35/layer (x~600 over 18 layers), so bf16/f32r anywhere in the
    state/message/edge-weight path blows the 2e-2 budget.
  - Per layer: m = x @ W on the tensor engine (node-major, banked 4 chunks
    per PSUM bank); TWO pipelined AllGathers build the message table in DRAM
    (table A = first 4095 rows of every core -> fires ~60% into the previous
    layer; table B = the rest).  dma_gather pulls edge rows; one 128x64
    matmul per chunk (selector = ew-scaled one-hot generated on-device by
    DVE) accumulates agg^T per half in its own PSUM bank; halves combine on
    ACT+DVE.  hi-gathers are issued one group late so the lo stream keeps
    the DMA engines busy until AllGather-B lands.
  - GRU feature-major; x^T resident in SBUF across all 18 layers.
"""
import os
import numpy as np

import concourse.bacc as bacc
import concourse.mybir as mybir
import concourse.tile as tile
from concourse import bass_utils

N = 50000
C = 128
NCORES = 8
NPC = N // NCORES            # 6250
WIN = 64                     # dst nodes per selector window
GRP = 8                      # windows per 512-col PSUM group
CHUNK = 128                  # edges per selector matmul
HL = 4095                    # per-core split: table A = locals [0,HL) of
                             # every core (8*4095=32760 rows, int16-safe),
                             # table B = the rest (8*2155=17240 rows)
NWIN = (NPC + WIN - 1) // WIN            # 98
NGRP = (NWIN + GRP - 1) // GRP           # 13
NLAYERS = 18
SUB = 16                    # max chunks per gather instruction / G tile

F32 = mybir.dt.float32
I16 = mybir.dt.int16

# 0: stream sel from DRAM; 2: batched on-device gen (DVE)
SEL_MODE = int(os.environ.get("K_SEL_ONDEV", "2"))
SEL_ONDEV = SEL_MODE > 0
LAG = int(os.environ.get("K_LAG", "1"))   # groups of lo-lead before hi issue


# --------------------------------------------------------------------------
# host-side preprocessing
# --------------------------------------------------------------------------

def preprocess(edge_index, edge_attr):
    src = np.asarray(edge_index[0], dtype=np.int64)
    dst = np.asarray(edge_index[1], dtype=np.int64)
    ew = np.asarray(edge_attr, dtype=np.float32)

    owner = dst // NPC
    dst_local = dst - owner * NPC
    win = dst_local // WIN
    src_owner = src // NPC
    src_local = src - src_owner * NPC
    half = (src_local >= HL).astype(np.int64)
    tab_idx = np.where(half == 0, src_owner * HL + src_local,
                       src_owner * (NPC - HL) + (src_local - HL))

    counts = np.zeros((NCORES, NWIN, 2), dtype=np.int64)
    np.add.at(counts, (owner, win, half), 1)
    K = (counts.max(axis=0) + CHUNK - 1) // CHUNK          # [NWIN, 2]
    # Every (window, half) needs >= 1 chunk: each half accumulates in its own
    # PSUM bank and agg = lo + hi, so every column must be written in both.
    K = np.maximum(K, 1)

    order = np.lexsort((half, win, owner))
    ssrc = tab_idx[order]
    sdl = dst_local[order]
    sew = ew[order]
    starts = {}
    pos = 0
    for c in range(NCORES):
        for w in range(NWIN):
            for h in range(2):
                n = int(counts[c, w, h])
                starts[(c, w, h)] = (pos, pos + n)
                pos += n

    schedule = []           # per group: (n_lo, n_hi, chunk_win list)
    total_chunks = 0
    for g in range(NGRP):
        wlo, whi = g * GRP, min((g + 1) * GRP, NWIN)
        chunk_win = []
        n_lo = n_hi = 0
        for w in range(wlo, whi):
            for _ in range(int(K[w, 0])):
                chunk_win.append(w - wlo)
                n_lo += 1
        for w in range(wlo, whi):
            for _ in range(int(K[w, 1])):
                chunk_win.append(w - wlo)
                n_hi += 1
        schedule.append((n_lo, n_hi, chunk_win))
        total_chunks += n_lo + n_hi

    n_lo_tot = sum(s[0] for s in schedule)
    n_hi_tot = sum(s[1] for s in schedule)

    per_core = []
    for c in range(NCORES):
        lo_idx = np.zeros(max(n_lo_tot, 1) * CHUNK, dtype=np.int16)
        hi_idx = np.zeros(max(n_hi_tot, 1) * CHUNK, dtype=np.int16)
        # compact selector encoding: per (slot, chunk) the dst column within
        # the window (or -1 for padding) and the edge weight.  The one-hot
        # selector tile is generated on-device as (iota == dcol) * ewc.
        dcol = np.full((CHUNK, total_chunks), -1.0, dtype=np.float32)
        ewc = np.zeros((CHUNK, total_chunks), dtype=np.float32)
        sel = (None if SEL_ONDEV else
               np.zeros((total_chunks, CHUNK, WIN), dtype=np.float32))
        ci = li = hi_i = 0
        for g in range(NGRP):
            wlo, whi = g * GRP, min((g + 1) * GRP, NWIN)
            for h in (0, 1):
                for w in range(wlo, whi):
                    a, b = starts[(c, w, h)]
                    es, ed, eww = ssrc[a:b], sdl[a:b], sew[a:b]
                    n = b - a
                    for k in range(int(K[w, h])):
                        s0, s1 = k * CHUNK, min((k + 1) * CHUNK, n)
                        cnt = max(0, s1 - s0)
                        if cnt > 0:
                            iv = es[s0:s1].astype(np.int16)
                            if h == 0:
                                lo_idx[li:li + cnt] = iv
                            else:
                                hi_idx[hi_i:hi_i + cnt] = iv
                            dcol[:cnt, ci] = ed[s0:s1] - w * WIN
                            ewc[:cnt, ci] = eww[s0:s1]
                            if sel is not None:
                                sel[ci, np.arange(cnt), ed[s0:s1] - w * WIN] = \
                                    eww[s0:s1]
                        if h == 0:
                            li += CHUNK
                        else:
                            hi_i += CHUNK
                        ci += 1

        def wrap(flat):
            ncols = len(flat) // 16
            out = np.empty((128, ncols), dtype=np.int16)
            v = flat.reshape(ncols, 16).T
            for g8 in range(8):
                out[g8 * 16:(g8 + 1) * 16] = v
            return out

        ent = dict(idx_lo=wrap(lo_idx), idx_hi=wrap(hi_idx))
        if SEL_ONDEV:
            ent.update(dcol=dcol, ewc=ewc)
        else:
            ent["sel"] = np.ascontiguousarray(
                sel.transpose(1, 0, 2).reshape(CHUNK, total_chunks * WIN))
        per_core.append(ent)
    return schedule, per_core


def make_inmaps(inp, per_core):
    x = np.asarray(inp["x"], dtype=np.float32)
    wm, wg, gb = _pack_params(inp)
    in_maps = []
    for c in range(NCORES):
        pc = per_core[c]
        m = {
            "xT_in": np.ascontiguousarray(x[c * NPC:(c + 1) * NPC].T),
            "idx_lo": pc["idx_lo"], "idx_hi": pc["idx_hi"],
            "wm": wm, "wg": wg, "gb": gb,
        }
        if SEL_ONDEV:
            m["dcol"], m["ewc"] = pc["dcol"], pc["ewc"]
        else:
            m["sel"] = pc["sel"]
        in_maps.append(m)
    return in_maps


# --------------------------------------------------------------------------
# program builder
# --------------------------------------------------------------------------

def build_program(schedule, n_layers=NLAYERS, stage=99):
    total_chunks = sum(s[0] + s[1] for s in schedule)
    n_lo_tot = sum(s[0] for s in schedule)
    n_hi_tot = sum(s[1] for s in schedule)

    nc = bacc.Bacc("TRN2", target_bir_lowering=False, debug=False,
                   num_devices=NCORES, num_swdge_queues=2)

    xT_in = nc.dram_tensor("xT_in", [128, NPC], F32, kind="ExternalInput")
    idxlo_in = nc.dram_tensor("idx_lo", [128, max(n_lo_tot, 1) * 8], I16, kind="ExternalInput")
    idxhi_in = nc.dram_tensor("idx_hi", [128, max(n_hi_tot, 1) * 8], I16, kind="ExternalInput")
    if SEL_ONDEV:
        dcol_in = nc.dram_tensor("dcol", [128, total_chunks], F32, kind="ExternalInput")
        ewc_in = nc.dram_tensor("ewc", [128, total_chunks], F32, kind="ExternalInput")
    else:
        sel_in = nc.dram_tensor("sel", [128, total_chunks * WIN], F32, kind="ExternalInput")
    wm_in = nc.dram_tensor("wm", [128, 4 * 128], F32, kind="ExternalInput")
    wg_in = nc.dram_tensor("wg", [128, 12 * 128], F32, kind="ExternalInput")
    gb_in = nc.dram_tensor("gb", [128, 8], F32, kind="ExternalInput")
    outT = nc.dram_tensor("outT", [128, NPC], F32, kind="ExternalOutput")

    RA, RB = NCORES * HL, NCORES * (NPC - HL)
    m_own = [nc.dram_tensor(f"m_own{i}", [NPC, C], F32) for i in range(2)]
    m_fullA = [nc.dram_tensor(f"m_fullA{i}", [RA, C], F32, addr_space="Shared")
               for i in range(2)]
    m_fullB = [nc.dram_tensor(f"m_fullB{i}", [RB, C], F32, addr_space="Shared")
               for i in range(2)]

    KA = (HL + 127) // 128          # 32 m chunks cover table A rows
    NKCH = (NPC + 127) // 128       # 49 m chunks total
    NB = (NKCH + 3) // 4            # 13 m banks (4 chunks each)

    with tile.TileContext(nc) as tc:
        with (
            tc.tile_pool(name="res", bufs=1) as res,
            tc.tile_pool(name="gpool", bufs=8) as gpool,
            tc.tile_pool(name="spool", bufs=8) as spool,
            tc.tile_pool(name="aggp", bufs=2, space="PSUM") as aggp,
            tc.tile_pool(name="gatep", bufs=4, space="PSUM") as gatep,
            tc.tile_pool(name="mmp", bufs=2, space="PSUM") as mmp,
            tc.tile_pool(name="asb", bufs=2) as asb,
            tc.tile_pool(name="tsb", bufs=10) as tsb,
            tc.tile_pool(name="msb", bufs=3) as msb,
        ):
            # resident tiles
            xT = res.tile([128, NPC], F32)
            idxlo = res.tile([128, max(n_lo_tot, 1) * 8], I16)
            idxhi = res.tile([128, max(n_hi_tot, 1) * 8], I16)
            wm = res.tile([128, 4 * 128], F32)
            wg = res.tile([128, 12 * 128], F32)
            gb = res.tile([128, 8], F32)
            nc.sync.dma_start(xT[:], xT_in[:])
            nc.sync.dma_start(idxlo[:], idxlo_in[:])
            nc.sync.dma_start(idxhi[:], idxhi_in[:])
            nc.sync.dma_start(wm[:], wm_in[:])
            nc.sync.dma_start(wg[:], wg_in[:])
            nc.sync.dma_start(gb[:], gb_in[:])
            if SEL_ONDEV:
                dcol = res.tile([128, total_chunks], F32)
                ewc = res.tile([128, total_chunks], F32)
                iota = res.tile([128, WIN], F32)
                nc.sync.dma_start(dcol[:], dcol_in[:])
                nc.sync.dma_start(ewc[:], ewc_in[:])
                nc.gpsimd.iota(iota[:], pattern=[[1, WIN]], base=0,
                               channel_multiplier=0,
                               allow_small_or_imprecise_dtypes=True)

            gcall = [0]   # gather-call counter (queue = gcall % 2)

            for L in range(n_layers):
                conv = 0 if L < 2 else 1
                wcol = (conv * 2 + (L % 2)) * 128
                sblk = conv * 6 * 128
                bcol = conv * 4
                relu = (L % 2 == 1) and (L < 17)
                mbufA = m_fullA[L % 2]
                mbufB = m_fullB[L % 2]
                mo = m_own[L % 2]

                # ---- m-phase: m_own = x_own @ W, banked 4 chunks per PSUM
                # bank -> one scalar copy + one DMA per 512 nodes. ----
                def m_bank(b):
                    # 4 full 128-node chunks share one PSUM bank, then one
                    # scalar copy + one DMA move 512 rows; the trailing
                    # partial chunk (rows 6144..6250) goes through the
                    # single-chunk path.
                    k0, k1 = b * 4, min((b + 1) * 4, NKCH)
                    full = [k for k in range(k0, k1) if (k + 1) * 128 <= NPC]
                    p = mmp.tile([128, 512], F32, tag="mm")
                    for k in full:
                        q = (k - k0) * 128
                        nc.tensor.matmul(p[:, q:q + 128],
                                         xT[:, k * 128:(k + 1) * 128],
                                         wm[:, wcol:wcol + 128],
                                         start=True, stop=True)
                    if full:
                        nf = len(full) * 128
                        s = msb.tile([128, 512], F32, tag="ms")
                        nc.scalar.copy(s[:, :nf], p[:, :nf])
                        r0 = k0 * 128
                        # DRAM row r0 + a*128 + prt <- s[prt, a*128 + col]
                        nc.sync.dma_start(
                            mo[r0:r0 + nf, :].rearrange(
                                "(a p) b -> p a b", p=128),
                            s[:, :nf].rearrange("p (a b) -> p a b", b=128))
                    for k in range(k0, k1):
                        if k in full:
                            continue
                        c0, c1 = k * 128, min((k + 1) * 128, NPC)
                        q = (k - k0) * 128
                        nc.tensor.matmul(p[:c1 - c0, q:q + 128],
                                         xT[:, c0:c1],
                                         wm[:, wcol:wcol + 128],
                                         start=True, stop=True)
                        s2 = msb.tile([128, 512], F32, tag="ms")
                        nc.scalar.copy(s2[:c1 - c0, :128],
                                       p[:c1 - c0, q:q + 128])
                        nc.sync.dma_start(mo[c0:c1, :], s2[:c1 - c0, :128])

                for b in range((KA + 3) // 4):          # banks 0..7 cover A
                    m_bank(b)
                if stage != 30:
                    nc.gpsimd.collective_compute(
                        "AllGather", mybir.AluOpType.bypass,
                        replica_groups=[list(range(NCORES))],
                        ins=[mo[0:HL, :]], outs=[mbufA[:]],
                    )
                for b in range((KA + 3) // 4, NB):
                    m_bank(b)
                if stage != 30:
                    nc.gpsimd.collective_compute(
                        "AllGather", mybir.AluOpType.bypass,
                        replica_groups=[list(range(NCORES))],
                        ins=[mo[HL:NPC, :]], outs=[mbufB[:]],
                    )
                m_lo = mbufA[:]
                m_hi = mbufB[:]
                if stage < 20:
                    continue

                # ---- gather + selector matmuls + GRU, software-pipelined:
                # lo(g) issues LAG groups ahead of hi(g). ----
                li = 0
                hi_i = 0
                # precompute per-group chunk-column bases (gather-stream
                # order: per group, lo chunks first then hi chunks)
                base = 0
                gbase = []
                for g in range(NGRP):
                    n_lo, n_hi, _ = schedule[g]
                    gbase.append((base, base + n_lo))
                    base += n_lo + n_hi

                lo_tiles = {}   # g -> list of (gt, st, nch)
                hi_tiles = {}

                def issue_gathers(g, h):
                    nonlocal li, hi_i
                    n_lo, n_hi, _ = schedule[g]
                    total = n_lo if h == 0 else n_hi
                    idx_t = idxlo if h == 0 else idxhi
                    table = m_lo if h == 0 else m_hi
                    cbase = gbase[g][0] if h == 0 else gbase[g][1]
                    tiles = []
                    j = 0
                    while j < total:
                        nch = min(SUB, total - j)
                        cur = li if h == 0 else hi_i
                        gt = gpool.tile([128, SUB * 128], F32, tag="g")
                        nc.gpsimd.dma_gather(
                            out_ap=gt[:, :nch * 128].rearrange(
                                "p (a b) -> p a b", b=128),
                            in_ap=table,
                            idxs_ap=idx_t[:, cur * 8:(cur + nch) * 8],
                            num_idxs=nch * 128, num_idxs_reg=nch * 128,
                            elem_size=C, single_packet=False,
                            queue_num=gcall[0] % 2,
                        )
                        gcall[0] += 1
                        st = spool.tile([128, SUB * WIN], F32, tag="s")
                        if stage >= 21:
                            c0 = cbase + j
                            if SEL_ONDEV:
                                io3 = iota[:].rearrange(
                                    "p (o w) -> p o w", o=1
                                ).broadcast_to([128, nch, WIN])
                                dc3 = dcol[:, c0:c0 + nch].rearrange(
                                    "p (n o) -> p n o", o=1
                                ).broadcast_to([128, nch, WIN])
                                ew3 = ewc[:, c0:c0 + nch].rearrange(
                                    "p (n o) -> p n o", o=1
                                ).broadcast_to([128, nch, WIN])
                                st3 = st[:, :nch * WIN].rearrange(
                                    "p (n w) -> p n w", w=WIN)
                                nc.vector.tensor_tensor(
                                    st3, io3, dc3, mybir.AluOpType.is_equal)
                                nc.vector.tensor_tensor(
                                    st3, st3, ew3, mybir.AluOpType.mult)
                            else:
                                nc.sync.dma_start(
                                    st[:, :nch * WIN],
                                    sel_in[:, c0 * WIN:(c0 + nch) * WIN])
                        tiles.append((gt, st, nch))
                        if h == 0:
                            li += nch
                        else:
                            hi_i += nch
                        j += nch
                    return tiles

                def mm_half(g, h, agg):
                    n_lo, n_hi, chunk_win = schedule[g]
                    cws = chunk_win[:n_lo] if h == 0 else chunk_win[n_lo:]
                    tiles = lo_tiles[g] if h == 0 else hi_tiles[g]
                    ti = 0
                    q = 0
                    for i, wg_i in enumerate(cws):
                        first = (i == 0) or cws[i - 1] != wg_i
                        lastc = (i == len(cws) - 1) or cws[i + 1] != wg_i
                        gt, st, nch = tiles[ti]
                        nc.tensor.matmul(
                            agg[:, wg_i * WIN:(wg_i + 1) * WIN],
                            gt[:, q * 128:(q + 1) * 128],
                            st[:, q * WIN:(q + 1) * WIN],
                            start=first, stop=lastc,
                        )
                        q += 1
                        if q == nch:
                            ti += 1
                            q = 0

                def process_group(g):
                    g0 = g * GRP * WIN
                    gw = min(GRP * WIN, NPC - g0)
                    agg_lo = aggp.tile([128, 512], F32, tag="agg")
                    agg_hi = aggp.tile([128, 512], F32, tag="agg")
                    if stage >= 22:
                        mm_half(g, 0, agg_lo)
                        mm_half(g, 1, agg_hi)
                    if stage < 23:
                        return
                    aggf = asb.tile([128, 512], F32, tag="aggf")
                    nc.scalar.copy(aggf[:, :gw], agg_hi[:, :gw])
                    aggs = asb.tile([128, 512], F32, tag="aggs")
                    nc.vector.tensor_add(aggs[:, :gw], aggf[:, :gw],
                                         agg_lo[:, :gw])
                    if stage < 24:
                        return
                    xg = xT[:, g0:g0 + gw]

                    def gate_mm(idx_ih, idx_hh, acc_two):
                        pt = gatep.tile([128, 512], F32, tag="gate")
                        nc.tensor.matmul(
                            pt[:, :gw],
                            wg[:, sblk + idx_ih * 128:sblk + (idx_ih + 1) * 128],
                            aggs[:, :gw], start=True, stop=not acc_two)
                        if acc_two:
                            nc.tensor.matmul(
                                pt[:, :gw],
                                wg[:, sblk + idx_hh * 128:sblk + (idx_hh + 1) * 128],
                                xg, start=False, stop=True)
                        return pt

                    r_pre = gate_mm(0, 3, True)
                    z_pre = gate_mm(1, 4, True)
                    i_n = gate_mm(2, None, False)
                    h_n = gatep.tile([128, 512], F32, tag="gate")
                    nc.tensor.matmul(h_n[:, :gw],
                                     wg[:, sblk + 5 * 128:sblk + 6 * 128],
                                     xg, start=True, stop=True)
                    if stage < 25:
                        return
                    r = tsb.tile([128, 512], F32, tag="t")
                    nc.scalar.activation(r[:, :gw], r_pre[:, :gw],
                                         mybir.ActivationFunctionType.Sigmoid,
                                         bias=gb[:, bcol + 0:bcol + 1])
                    z = tsb.tile([128, 512], F32, tag="t")
                    nc.scalar.activation(z[:, :gw], z_pre[:, :gw],
                                         mybir.ActivationFunctionType.Sigmoid,
                                         bias=gb[:, bcol + 1:bcol + 2])
                    rh = tsb.tile([128, 512], F32, tag="t")
                    nc.vector.scalar_tensor_tensor(
                        rh[:, :gw], h_n[:, :gw], gb[:, bcol + 3:bcol + 4],
                        r[:, :gw], mybir.AluOpType.add, mybir.AluOpType.mult)
                    t1 = tsb.tile([128, 512], F32, tag="t")
                    nc.vector.tensor_add(t1[:, :gw], i_n[:, :gw], rh[:, :gw])
                    n_t = tsb.tile([128, 512], F32, tag="t")
                    nc.scalar.activation(n_t[:, :gw], t1[:, :gw],
                                         mybir.ActivationFunctionType.Tanh,
                                         bias=gb[:, bcol + 2:bcol + 3])
                    d = tsb.tile([128, 512], F32, tag="t")
                    nc.vector.tensor_sub(d[:, :gw], xg, n_t[:, :gw])
                    zd = tsb.tile([128, 512], F32, tag="t")
                    nc.vector.tensor_mul(zd[:, :gw], z[:, :gw], d[:, :gw])
                    nc.vector.tensor_add(xg, n_t[:, :gw], zd[:, :gw])
                    if relu:
                        nc.vector.tensor_scalar_max(xg, xg, 0.0)

                # software pipeline: lo(g) ... lo(g+LAG) | hi(g) | process(g)
                for g in range(NGRP + LAG):
                    if g < NGRP:
                        lo_tiles[g] = issue_gathers(g, 0)
                    gp = g - LAG
                    if 0 <= gp < NGRP:
                        hi_tiles[gp] = issue_gathers(gp, 1)
                        process_group(gp)
                        del lo_tiles[gp], hi_tiles[gp]

            nc.sync.dma_start(outT[:], xT[:])

    nc.compile()
    return nc


# --------------------------------------------------------------------------
# entry point
# --------------------------------------------------------------------------

def _pack_params(inputs):
    wm = np.zeros((128, 4 * 128), dtype=np.float32)
    wg = np.zeros((128, 12 * 128), dtype=np.float32)
    gb = np.zeros((128, 8), dtype=np.float32)
    for conv, tag in ((0, "1"), (1, "2")):
        w = np.asarray(inputs[f"w{tag}"], dtype=np.float32)
        wih = np.asarray(inputs[f"wih{tag}"], dtype=np.float32)
        whh = np.asarray(inputs[f"whh{tag}"], dtype=np.float32)
        bih = np.asarray(inputs[f"bih{tag}"], dtype=np.float32)
        bhh = np.asarray(inputs[f"bhh{tag}"], dtype=np.float32)
        for l in range(2):
            wm[:, (conv * 2 + l) * 128:(conv * 2 + l + 1) * 128] = w[l]
        for i, mat in enumerate((wih[0:128], wih[128:256], wih[256:384],
                                 whh[0:128], whh[128:256], whh[256:384])):
            wg[:, (conv * 6 + i) * 128:(conv * 6 + i + 1) * 128] = mat.T
        gb[:, conv * 4 + 0] = bih[0:128] + bhh[0:128]
        gb[:, conv * 4 + 1] = bih[128:256] + bhh[128:256]
        gb[:, conv * 4 + 2] = bih[256:384]
        gb[:, conv * 4 + 3] = bhh[256:384]
    return wm, wg, gb


_CACHE = {}


def kernel(**inputs):
    schedule, per_core = preprocess(inputs["edge_index"], inputs["edge_attr"])

    key = tuple((s[0], s[1]) for s in schedule)
    if key not in _CACHE:
        _CACHE[key] = build_program(schedule)
    nc = _CACHE[key]

    in_maps = make_inmaps(inputs, per_core)
    res = bass_utils.run_bass_kernel_spmd(nc, in_maps, list(range(NCORES)))
    out = np.concatenate(
        [res.results[c]["outT"].T for c in range(NCORES)], axis=0)
    return out.astype(np.float32)


# revision 3
# speedup vs baseline: 1.1984x; 1.1984x over previous
"""Trainium2 Bass kernel for a 2-layer GatedGraphConv encoder (9 convs, 18
message-passing + GRU steps) on N=50000 nodes, E=800000 edges, C=128.

Strategy (8 NeuronCores, SPMD single program):
  - Nodes are block-sharded: core c owns dst rows [c*6250, (c+1)*6250).
  - Everything stays fp32: the GRU iteration amplifies injected noise
    ~x1.35/layer (x~600 over 18 layers), so bf16/f32r anywhere in the
    state/message/edge-weight path blows the 2e-2 budget.
  - Per layer: m = x @ W on the tensor engine (node-major, banked 4 chunks
    per PSUM bank); TWO pipelined AllGathers build the message table in DRAM
    (table A = first 4095 rows of every core -> fires ~60% into the previous
    layer; table B = the rest).  dma_gather pulls edge rows; one 128x64
    matmul per chunk (selector = ew-scaled one-hot generated on-device by
    DVE) accumulates agg^T per half in its own PSUM bank; halves combine on
    ACT+DVE.  hi-gathers are issued one group late so the lo stream keeps
    the DMA engines busy until AllGather-B lands.
  - GRU feature-major; x^T resident in SBUF across all 18 layers.
"""
import os
import numpy as np

import concourse.bacc as bacc
import concourse.mybir as mybir
import concourse.tile as tile
from concourse import bass_utils

N = 50000
C = 128
NCORES = 8
NPC = N // NCORES            # 6250
WIN = 64                     # dst nodes per selector window
GRP = 8                      # windows per 512-col PSUM group
CHUNK = 128                  # edges per selector matmul
HL = 4095                    # per-core split: table A = locals [0,HL) of
                             # every core (8*4095=32760 rows, int16-safe),
                             # table B = the rest (8*2155=17240 rows)
NWIN = (NPC + WIN - 1) // WIN            # 98
NGRP = (NWIN + GRP - 1) // GRP           # 13
NLAYERS = 18
SUB = 16                    # max chunks per gather instruction / G tile

F32 = mybir.dt.float32
I16 = mybir.dt.int16

# 0: stream sel from DRAM; 2: batched on-device gen (DVE)
SEL_MODE = int(os.environ.get("K_SEL_ONDEV", "2"))
SEL_ONDEV = SEL_MODE > 0
LAG = int(os.environ.get("K_LAG", "1"))   # groups of lo-lead before hi issue


# --------------------------------------------------------------------------
# host-side preprocessing
# --------------------------------------------------------------------------

def preprocess(edge_index, edge_attr):
    src = np.asarray(edge_index[0], dtype=np.int64)
    dst = np.asarray(edge_index[1], dtype=np.int64)
    ew = np.asarray(edge_attr, dtype=np.float32)

    owner = dst // NPC
    dst_local = dst - owner * NPC
    win = dst_local // WIN
    src_owner = src // NPC
    src_local = src - src_owner * NPC
    half = (src_local >= HL).astype(np.int64)
    tab_idx = np.where(half == 0, src_owner * HL + src_local,
                       src_owner * (NPC - HL) + (src_local - HL))

    counts = np.zeros((NCORES, NWIN, 2), dtype=np.int64)
    np.add.at(counts, (owner, win, half), 1)
    K = (counts.max(axis=0) + CHUNK - 1) // CHUNK          # [NWIN, 2]
    # Every (window, half) needs >= 1 chunk: each half accumulates in its own
    # PSUM bank and agg = lo + hi, so every column must be written in both.
    K = np.maximum(K, 1)

    order = np.lexsort((half, win, owner))
    ssrc = tab_idx[order]
    sdl = dst_local[order]
    sew = ew[order]
    starts = {}
    pos = 0
    for c in range(NCORES):
        for w in range(NWIN):
            for h in range(2):
                n = int(counts[c, w, h])
                starts[(c, w, h)] = (pos, pos + n)
                pos += n

    schedule = []           # per group: (n_lo, n_hi, chunk_win list)
    total_chunks = 0
    for g in range(NGRP):
        wlo, whi = g * GRP, min((g + 1) * GRP, NWIN)
        chunk_win = []
        n_lo = n_hi = 0
        for w in range(wlo, whi):
            for _ in range(int(K[w, 0])):
                chunk_win.append(w - wlo)
                n_lo += 1
        for w in range(wlo, whi):
            for _ in range(int(K[w, 1])):
                chunk_win.append(w - wlo)
                n_hi += 1
        schedule.append((n_lo, n_hi, chunk_win))
        total_chunks += n_lo + n_hi

    n_lo_tot = sum(s[0] for s in schedule)
    n_hi_tot = sum(s[1] for s in schedule)

    per_core = []
    for c in range(NCORES):
        lo_idx = np.zeros(max(n_lo_tot, 1) * CHUNK, dtype=np.int16)
        hi_idx = np.zeros(max(n_hi_tot, 1) * CHUNK, dtype=np.int16)
        # compact selector encoding: per (slot, chunk) the dst column within
        # the window (or -1 for padding) and the edge weight.  The one-hot
        # selector tile is generated on-device as (iota == dcol) * ewc.
        dcol = np.full((CHUNK, total_chunks), -1.0, dtype=np.float32)
        ewc = np.zeros((CHUNK, total_chunks), dtype=np.float32)
        sel = (None if SEL_ONDEV else
               np.zeros((total_chunks, CHUNK, WIN), dtype=np.float32))
        ci = li = hi_i = 0
        for g in range(NGRP):
            wlo, whi = g * GRP, min((g + 1) * GRP, NWIN)
            for h in (0, 1):
                for w in range(wlo, whi):
                    a, b = starts[(c, w, h)]
                    es, ed, eww = ssrc[a:b], sdl[a:b], sew[a:b]
                    n = b - a
                    for k in range(int(K[w, h])):
                        s0, s1 = k * CHUNK, min((k + 1) * CHUNK, n)
                        cnt = max(0, s1 - s0)
                        if cnt > 0:
                            iv = es[s0:s1].astype(np.int16)
                            if h == 0:
                                lo_idx[li:li + cnt] = iv
                            else:
                                hi_idx[hi_i:hi_i + cnt] = iv
                            dcol[:cnt, ci] = ed[s0:s1] - w * WIN
                            ewc[:cnt, ci] = eww[s0:s1]
                            if sel is not None:
                                sel[ci, np.arange(cnt), ed[s0:s1] - w * WIN] = \
                                    eww[s0:s1]
                        if h == 0:
                            li += CHUNK
                        else:
                            hi_i += CHUNK
                        ci += 1

        def wrap(flat):
            ncols = len(flat) // 16
            out = np.empty((128, ncols), dtype=np.int16)
            v = flat.reshape(ncols, 16).T
            for g8 in range(8):
                out[g8 * 16:(g8 + 1) * 16] = v
            return out

        ent = dict(idx_lo=wrap(lo_idx), idx_hi=wrap(hi_idx))
        if SEL_ONDEV:
            ent.update(dcol=dcol, ewc=ewc)
        else:
            ent["sel"] = np.ascontiguousarray(
                sel.transpose(1, 0, 2).reshape(CHUNK, total_chunks * WIN))
        per_core.append(ent)
    return schedule, per_core


def make_inmaps(inp, per_core):
    x = np.asarray(inp["x"], dtype=np.float32)
    wm, wg, gb = _pack_params(inp)
    in_maps = []
    for c in range(NCORES):
        pc = per_core[c]
        m = {
            "xT_in": np.ascontiguousarray(x[c * NPC:(c + 1) * NPC].T),
            "idx_lo": pc["idx_lo"], "idx_hi": pc["idx_hi"],
            "wm": wm, "wg": wg, "gb": gb,
        }
        if SEL_ONDEV:
            m["dcol"], m["ewc"] = pc["dcol"], pc["ewc"]
        else:
            m["sel"] = pc["sel"]
        in_maps.append(m)
    return in_maps


# --------------------------------------------------------------------------
# program builder
# --------------------------------------------------------------------------

def build_program(schedule, n_layers=NLAYERS, stage=99):
    total_chunks = sum(s[0] + s[1] for s in schedule)
    n_lo_tot = sum(s[0] for s in schedule)
    n_hi_tot = sum(s[1] for s in schedule)

    nc = bacc.Bacc("TRN2", target_bir_lowering=False, debug=False,
                   num_devices=NCORES, num_swdge_queues=2)

    xT_in = nc.dram_tensor("xT_in", [128, NPC], F32, kind="ExternalInput")
    idxlo_in = nc.dram_tensor("idx_lo", [128, max(n_lo_tot, 1) * 8], I16, kind="ExternalInput")
    idxhi_in = nc.dram_tensor("idx_hi", [128, max(n_hi_tot, 1) * 8], I16, kind="ExternalInput")
    if SEL_ONDEV:
        dcol_in = nc.dram_tensor("dcol", [128, total_chunks], F32, kind="ExternalInput")
        ewc_in = nc.dram_tensor("ewc", [128, total_chunks], F32, kind="ExternalInput")
    else:
        sel_in = nc.dram_tensor("sel", [128, total_chunks * WIN], F32, kind="ExternalInput")
    wm_in = nc.dram_tensor("wm", [128, 4 * 128], F32, kind="ExternalInput")
    wg_in = nc.dram_tensor("wg", [128, 12 * 128], F32, kind="ExternalInput")
    gb_in = nc.dram_tensor("gb", [128, 8], F32, kind="ExternalInput")
    outT = nc.dram_tensor("outT", [128, NPC], F32, kind="ExternalOutput")

    RA, RB = NCORES * HL, NCORES * (NPC - HL)
    m_own = [nc.dram_tensor(f"m_own{i}", [NPC, C], F32) for i in range(2)]
    m_fullA = [nc.dram_tensor(f"m_fullA{i}", [RA, C], F32, addr_space="Shared")
               for i in range(2)]
    m_fullB = [nc.dram_tensor(f"m_fullB{i}", [RB, C], F32, addr_space="Shared")
               for i in range(2)]

    KA = (HL + 127) // 128          # 32 m chunks cover table A rows
    NKCH = (NPC + 127) // 128       # 49 m chunks total
    NB = (NKCH + 3) // 4            # 13 m banks (4 chunks each)

    with tile.TileContext(nc) as tc:
        with (
            tc.tile_pool(name="res", bufs=1) as res,
            tc.tile_pool(name="gpool", bufs=8) as gpool,
            tc.tile_pool(name="spool", bufs=8) as spool,
            tc.tile_pool(name="aggp", bufs=2, space="PSUM") as aggp,
            tc.tile_pool(name="gatep", bufs=5, space="PSUM") as gatep,
            tc.tile_pool(name="mmp", bufs=1, space="PSUM") as mmp,
            tc.tile_pool(name="asb", bufs=3) as asb,
            tc.tile_pool(name="tsb", bufs=10) as tsb,
            tc.tile_pool(name="msb", bufs=3) as msb,
        ):
            # resident tiles
            xT = res.tile([128, NPC], F32)
            idxlo = res.tile([128, max(n_lo_tot, 1) * 8], I16)
            idxhi = res.tile([128, max(n_hi_tot, 1) * 8], I16)
            wm = res.tile([128, 4 * 128], F32)
            wg = res.tile([128, 12 * 128], F32)
            gb = res.tile([128, 8], F32)
            nc.sync.dma_start(xT[:], xT_in[:])
            nc.sync.dma_start(idxlo[:], idxlo_in[:])
            nc.sync.dma_start(idxhi[:], idxhi_in[:])
            nc.sync.dma_start(wm[:], wm_in[:])
            nc.sync.dma_start(wg[:], wg_in[:])
            nc.sync.dma_start(gb[:], gb_in[:])
            if SEL_ONDEV:
                dcol = res.tile([128, total_chunks], F32)
                ewc = res.tile([128, total_chunks], F32)
                iota = res.tile([128, WIN], F32)
                nc.sync.dma_start(dcol[:], dcol_in[:])
                nc.sync.dma_start(ewc[:], ewc_in[:])
                nc.gpsimd.iota(iota[:], pattern=[[1, WIN]], base=0,
                               channel_multiplier=0,
                               allow_small_or_imprecise_dtypes=True)

            gcall = [0]   # gather-call counter (queue = gcall % 2)

            for L in range(n_layers):
                conv = 0 if L < 2 else 1
                wcol = (conv * 2 + (L % 2)) * 128
                sblk = conv * 6 * 128
                bcol = conv * 4
                relu = (L % 2 == 1) and (L < 17)
                mbufA = m_fullA[L % 2]
                mbufB = m_fullB[L % 2]
                mo = m_own[L % 2]

                # ---- m-phase: m_own = x_own @ W, banked 4 chunks per PSUM
                # bank -> one scalar copy + one DMA per 512 nodes. ----
                def m_bank(b):
                    # 4 full 128-node chunks share one PSUM bank, then one
                    # scalar copy + one DMA move 512 rows; the trailing
                    # partial chunk (rows 6144..6250) goes through the
                    # single-chunk path.
                    k0, k1 = b * 4, min((b + 1) * 4, NKCH)
                    full = [k for k in range(k0, k1) if (k + 1) * 128 <= NPC]
                    p = mmp.tile([128, 512], F32, tag="mm")
                    for k in full:
                        q = (k - k0) * 128
                        nc.tensor.matmul(p[:, q:q + 128],
                                         xT[:, k * 128:(k + 1) * 128],
                                         wm[:, wcol:wcol + 128],
                                         start=True, stop=True)
                    if full:
                        nf = len(full) * 128
                        s = msb.tile([128, 512], F32, tag="ms")
                        nc.scalar.copy(s[:, :nf], p[:, :nf])
                        r0 = k0 * 128
                        # DRAM row r0 + a*128 + prt <- s[prt, a*128 + col]
                        nc.sync.dma_start(
                            mo[r0:r0 + nf, :].rearrange(
                                "(a p) b -> p a b", p=128),
                            s[:, :nf].rearrange("p (a b) -> p a b", b=128))
                    for k in range(k0, k1):
                        if k in full:
                            continue
                        c0, c1 = k * 128, min((k + 1) * 128, NPC)
                        q = (k - k0) * 128
                        nc.tensor.matmul(p[:c1 - c0, q:q + 128],
                                         xT[:, c0:c1],
                                         wm[:, wcol:wcol + 128],
                                         start=True, stop=True)
                        s2 = msb.tile([128, 512], F32, tag="ms")
                        nc.scalar.copy(s2[:c1 - c0, :128],
                                       p[:c1 - c0, q:q + 128])
                        nc.sync.dma_start(mo[c0:c1, :], s2[:c1 - c0, :128])

                for b in range((KA + 3) // 4):          # banks 0..7 cover A
                    m_bank(b)
                if stage != 30:
                    nc.gpsimd.collective_compute(
                        "AllGather", mybir.AluOpType.bypass,
                        replica_groups=[list(range(NCORES))],
                        ins=[mo[0:HL, :]], outs=[mbufA[:]],
                    )
                for b in range((KA + 3) // 4, NB):
                    m_bank(b)
                if stage != 30:
                    nc.gpsimd.collective_compute(
                        "AllGather", mybir.AluOpType.bypass,
                        replica_groups=[list(range(NCORES))],
                        ins=[mo[HL:NPC, :]], outs=[mbufB[:]],
                    )
                m_lo = mbufA[:]
                m_hi = mbufB[:]
                if stage < 20:
                    continue

                # ---- gather + selector matmuls + GRU, software-pipelined:
                # lo(g) issues LAG groups ahead of hi(g). ----
                li = 0
                hi_i = 0
                # precompute per-group chunk-column bases (gather-stream
                # order: per group, lo chunks first then hi chunks)
                base = 0
                gbase = []
                for g in range(NGRP):
                    n_lo, n_hi, _ = schedule[g]
                    gbase.append((base, base + n_lo))
                    base += n_lo + n_hi

                lo_tiles = {}   # g -> list of (gt, st, nch)
                hi_tiles = {}

                def issue_gathers(g, h):
                    nonlocal li, hi_i
                    n_lo, n_hi, _ = schedule[g]
                    total = n_lo if h == 0 else n_hi
                    idx_t = idxlo if h == 0 else idxhi
                    table = m_lo if h == 0 else m_hi
                    cbase = gbase[g][0] if h == 0 else gbase[g][1]
                    tiles = []
                    j = 0
                    while j < total:
                        nch = min(SUB, total - j)
                        cur = li if h == 0 else hi_i
                        gt = gpool.tile([128, SUB * 128], F32, tag="g")
                        nc.gpsimd.dma_gather(
                            out_ap=gt[:, :nch * 128].rearrange(
                                "p (a b) -> p a b", b=128),
                            in_ap=table,
                            idxs_ap=idx_t[:, cur * 8:(cur + nch) * 8],
                            num_idxs=nch * 128, num_idxs_reg=nch * 128,
                            elem_size=C, single_packet=False,
                            queue_num=gcall[0] % 2,
                        )
                        gcall[0] += 1
                        st = spool.tile([128, SUB * WIN], F32, tag="s")
                        if stage >= 21:
                            c0 = cbase + j
                            if SEL_ONDEV:
                                io3 = iota[:].rearrange(
                                    "p (o w) -> p o w", o=1
                                ).broadcast_to([128, nch, WIN])
                                dc3 = dcol[:, c0:c0 + nch].rearrange(
                                    "p (n o) -> p n o", o=1
                                ).broadcast_to([128, nch, WIN])
                                ew3 = ewc[:, c0:c0 + nch].rearrange(
                                    "p (n o) -> p n o", o=1
                                ).broadcast_to([128, nch, WIN])
                                st3 = st[:, :nch * WIN].rearrange(
                                    "p (n w) -> p n w", w=WIN)
                                nc.vector.tensor_tensor(
                                    st3, io3, dc3, mybir.AluOpType.is_equal)
                                nc.vector.tensor_tensor(
                                    st3, st3, ew3, mybir.AluOpType.mult)
                            else:
                                nc.sync.dma_start(
                                    st[:, :nch * WIN],
                                    sel_in[:, c0 * WIN:(c0 + nch) * WIN])
                        tiles.append((gt, st, nch))
                        if h == 0:
                            li += nch
                        else:
                            hi_i += nch
                        j += nch
                    return tiles

                def mm_half(g, h, agg):
                    n_lo, n_hi, chunk_win = schedule[g]
                    cws = chunk_win[:n_lo] if h == 0 else chunk_win[n_lo:]
                    tiles = lo_tiles[g] if h == 0 else hi_tiles[g]
                    ti = 0
                    q = 0
                    for i, wg_i in enumerate(cws):
                        first = (i == 0) or cws[i - 1] != wg_i
                        lastc = (i == len(cws) - 1) or cws[i + 1] != wg_i
                        gt, st, nch = tiles[ti]
                        nc.tensor.matmul(
                            agg[:, wg_i * WIN:(wg_i + 1) * WIN],
                            gt[:, q * 128:(q + 1) * 128],
                            st[:, q * WIN:(q + 1) * WIN],
                            start=first, stop=lastc,
                        )
                        q += 1
                        if q == nch:
                            ti += 1
                            q = 0

                def process_group(g):
                    g0 = g * GRP * WIN
                    gw = min(GRP * WIN, NPC - g0)
                    agg_lo = aggp.tile([128, 512], F32, tag="agg")
                    agg_hi = aggp.tile([128, 512], F32, tag="agg")
                    if stage >= 22:
                        mm_half(g, 0, agg_lo)
                        mm_half(g, 1, agg_hi)
                    if stage < 23:
                        return
                    aggf = asb.tile([128, 512], F32, tag="aggf")
                    nc.scalar.copy(aggf[:, :gw], agg_hi[:, :gw])
                    aggs = asb.tile([128, 512], F32, tag="aggs")
                    nc.vector.tensor_add(aggs[:, :gw], aggf[:, :gw],
                                         agg_lo[:, :gw])
                    if stage < 24:
                        return
                    xg = xT[:, g0:g0 + gw]

                    def gate_mm(idx_ih, idx_hh, acc_two):
                        pt = gatep.tile([128, 512], F32, tag="gate")
                        nc.tensor.matmul(
                            pt[:, :gw],
                            wg[:, sblk + idx_ih * 128:sblk + (idx_ih + 1) * 128],
                            aggs[:, :gw], start=True, stop=not acc_two)
                        if acc_two:
                            nc.tensor.matmul(
                                pt[:, :gw],
                                wg[:, sblk + idx_hh * 128:sblk + (idx_hh + 1) * 128],
                                xg, start=False, stop=True)
                        return pt

                    r_pre = gate_mm(0, 3, True)
                    z_pre = gate_mm(1, 4, True)
                    i_n = gate_mm(2, None, False)
                    h_n = gatep.tile([128, 512], F32, tag="gate")
                    nc.tensor.matmul(h_n[:, :gw],
                                     wg[:, sblk + 5 * 128:sblk + 6 * 128],
                                     xg, start=True, stop=True)
                    if stage < 25:
                        return
                    r = tsb.tile([128, 512], F32, tag="t")
                    nc.scalar.activation(r[:, :gw], r_pre[:, :gw],
                                         mybir.ActivationFunctionType.Sigmoid,
                                         bias=gb[:, bcol + 0:bcol + 1])
                    z = tsb.tile([128, 512], F32, tag="t")
                    nc.scalar.activation(z[:, :gw], z_pre[:, :gw],
                                         mybir.ActivationFunctionType.Sigmoid,
                                         bias=gb[:, bcol + 1:bcol + 2])
                    rh = tsb.tile([128, 512], F32, tag="t")
                    nc.vector.scalar_tensor_tensor(
                        rh[:, :gw], h_n[:, :gw], gb[:, bcol + 3:bcol + 4],
                        r[:, :gw], mybir.AluOpType.add, mybir.AluOpType.mult)
                    t1 = tsb.tile([128, 512], F32, tag="t")
                    nc.vector.tensor_add(t1[:, :gw], i_n[:, :gw], rh[:, :gw])
                    n_t = tsb.tile([128, 512], F32, tag="t")
                    nc.scalar.activation(n_t[:, :gw], t1[:, :gw],
                                         mybir.ActivationFunctionType.Tanh,
                                         bias=gb[:, bcol + 2:bcol + 3])
                    d = tsb.tile([128, 512], F32, tag="t")
                    nc.vector.tensor_sub(d[:, :gw], xg, n_t[:, :gw])
                    zd = tsb.tile([128, 512], F32, tag="t")
                    nc.vector.tensor_mul(zd[:, :gw], z[:, :gw], d[:, :gw])
                    nc.vector.tensor_add(xg, n_t[:, :gw], zd[:, :gw])
                    if relu:
                        nc.vector.tensor_scalar_max(xg, xg, 0.0)

                # software pipeline: lo(g) ... lo(g+LAG) | hi(g) | process(g)
                for g in range(NGRP + LAG):
                    if g < NGRP:
                        lo_tiles[g] = issue_gathers(g, 0)
                    gp = g - LAG
                    if 0 <= gp < NGRP:
                        hi_tiles[gp] = issue_gathers(gp, 1)
                        process_group(gp)
                        del lo_tiles[gp], hi_tiles[gp]

            nc.sync.dma_start(outT[:], xT[:])

    nc.compile()
    return nc


# --------------------------------------------------------------------------
# entry point
# --------------------------------------------------------------------------

def _pack_params(inputs):
    wm = np.zeros((128, 4 * 128), dtype=np.float32)
    wg = np.zeros((128, 12 * 128), dtype=np.float32)
    gb = np.zeros((128, 8), dtype=np.float32)
    for conv, tag in ((0, "1"), (1, "2")):
        w = np.asarray(inputs[f"w{tag}"], dtype=np.float32)
        wih = np.asarray(inputs[f"wih{tag}"], dtype=np.float32)
        whh = np.asarray(inputs[f"whh{tag}"], dtype=np.float32)
        bih = np.asarray(inputs[f"bih{tag}"], dtype=np.float32)
        bhh = np.asarray(inputs[f"bhh{tag}"], dtype=np.float32)
        for l in range(2):
            wm[:, (conv * 2 + l) * 128:(conv * 2 + l + 1) * 128] = w[l]
        for i, mat in enumerate((wih[0:128], wih[128:256], wih[256:384],
                                 whh[0:128], whh[128:256], whh[256:384])):
            wg[:, (conv * 6 + i) * 128:(conv * 6 + i + 1) * 128] = mat.T
        gb[:, conv * 4 + 0] = bih[0:128] + bhh[0:128]
        gb[:, conv * 4 + 1] = bih[128:256] + bhh[128:256]
        gb[:, conv * 4 + 2] = bih[256:384]
        gb[:, conv * 4 + 3] = bhh[256:384]
    return wm, wg, gb


_CACHE = {}


def kernel(**inputs):
    schedule, per_core = preprocess(inputs["edge_index"], inputs["edge_attr"])

    key = tuple((s[0], s[1]) for s in schedule)
    if key not in _CACHE:
        _CACHE[key] = build_program(schedule)
    nc = _CACHE[key]

    in_maps = make_inmaps(inputs, per_core)
    res = bass_utils.run_bass_kernel_spmd(nc, in_maps, list(range(NCORES)))
    out = np.concatenate(
        [res.results[c]["outT"].T for c in range(NCORES)], axis=0)
    return out.astype(np.float32)
